# revision 17
# baseline (speedup 1.0000x reference)
"""BoT block (conv1x1+BN+ReLU -> 4-head MHSA+posemb -> conv1x1+BN -> residual+ReLU)
on 8 trn2 NeuronCores, data-parallel over batch (2 images per core).

v2: fp8(e4m3) DoubleRow matmuls for conv1/qk/v/att/conv3 (2x PE throughput),
attention computed directly in [d, x] layout (no per-tile transposes),
softmax sums via a wide fp8 ones matmul (sum pre-broadcast to all
partitions), f16 output stores, fp8 x input.

Self-contained: hardcodes shapes N=16, Cin=2048, H=W=32, heads=4, dqk=dv=128.
"""
import numpy as np
import ml_dtypes

import concourse.bass as bass
import concourse.mybir as mybir
import concourse.tile as tile
from concourse import bacc
from concourse.bass_utils import run_bass_kernel_spmd

EPS = 1e-5
HEADS = 4
DQK = 128
DV = 128
SCALE = DQK ** -0.5
N_IMG = 16
CIN = 2048
H = W = 32
HW = H * W            # 1024
MID = HEADS * DV      # 512
NCORES = 8
IMGS_PER_CORE = N_IMG // NCORES  # 2

P = 128
F8 = mybir.dt.float8e4
F16 = mybir.dt.float16
F32 = mybir.dt.float32
AF = mybir.ActivationFunctionType
ALU = mybir.AluOpType
DR = mybir.MatmulPerfMode.DoubleRow

KT1 = CIN // P        # 16 k-tiles for conv1
OT1 = MID // P        # 4 out-tiles for conv1
KT2 = MID // P        # 4 k-tiles for qk/v/conv3
OT3 = CIN // P        # 16 out-tiles for conv3
YT = HW // P          # 8 y-tiles
NH = HW // 512        # 2 halves of 512

SW = 256.0            # fp8 weight scale
SA = 64.0             # fp8 attention-out scale (folded via reciprocal)
RES = SA * SW         # identity-matmul residual scale (2^14)

_BUILT = {}
NP_F8 = ml_dtypes.float8_e4m3


def _build():
    if "nc" in _BUILT:
        return _BUILT["nc"]
    nc = bacc.Bacc("TRN2", target_bir_lowering=False, debug=False,
                   num_devices=NCORES)

    # ---- DRAM I/O (per-core shard) ----
    x8_d = nc.dram_tensor("x8", [IMGS_PER_CORE, KT1, P, HW], F8, kind="ExternalInput")
    xr_d = nc.dram_tensor("xr", [IMGS_PER_CORE, KT1, P, HW], F16, kind="ExternalInput")
    w1t_d = nc.dram_tensor("w1t", [KT1, P, MID], F8, kind="ExternalInput")
    qkwt_d = nc.dram_tensor("qkwt", [KT2, P, 2 * MID], F8, kind="ExternalInput")
    vwt_d = nc.dram_tensor("vwt", [KT2, P, MID], F8, kind="ExternalInput")
    w3t_d = nc.dram_tensor("w3t", [KT2, P, CIN], F8, kind="ExternalInput")
    embt_d = nc.dram_tensor("embt", [P, HW], F32, kind="ExternalInput")
    b1_d = nc.dram_tensor("b1", [P, OT1], F32, kind="ExternalInput")
    out_d = nc.dram_tensor("out", [IMGS_PER_CORE, OT3, P, HW], F16,
                           kind="ExternalOutput")

    with tile.TileContext(nc) as tc:
        with (
            tc.tile_pool(name="consts", bufs=1) as consts,
            tc.tile_pool(name="xpool", bufs=2) as xpool,
            tc.tile_pool(name="feat", bufs=2) as featp,
            tc.tile_pool(name="qk", bufs=2) as qkp,
            tc.tile_pool(name="vaug", bufs=2) as vaugp,
            tc.tile_pool(name="et", bufs=5) as etp,
            tc.tile_pool(name="atf", bufs=2) as atfp,
            tc.tile_pool(name="rc", bufs=2) as rcp,
            tc.tile_pool(name="outp", bufs=3) as outp,
            tc.tile_pool(name="xres", bufs=16) as xresp,
            tc.tile_pool(name="ps_sml", bufs=4, space="PSUM") as ps_sml,
            tc.tile_pool(name="ps_big", bufs=2, space="PSUM") as ps_big,
        ):
            # ---- constants; w1t DMAs interleaved with first x loads ----
            w1t = consts.tile([P, KT1, MID], F8)
            b1 = consts.tile([P, OT1], F32)
            nc.sync.dma_start(b1[:], b1_d.ap())
            qkwt = consts.tile([P, KT2, 2 * MID], F8)
            vwt = consts.tile([P, KT2, MID], F8)
            w3t = consts.tile([P, KT2, CIN], F8)
            embt = consts.tile([P, HW], F32)
            ident = consts.tile([P, P], F16)
            ones2 = consts.tile([P, 2, P], F8)
            # identity scaled by RES: residual rides the conv3 psum
            nc.gpsimd.memset(ident[:], 0.0)
            nc.gpsimd.affine_select(
                out=ident[:], in_=ident[:],
                compare_op=ALU.not_equal, fill=RES, base=0,
                pattern=[[-1, P]], channel_multiplier=1)
            nc.vector.memset(ones2[:], 1.0)

            for i in range(IMGS_PER_CORE):
                # ---- conv1 (fp8 DR) + BN + ReLU -> feat [c=512, hw] f8 ----
                x8 = xpool.tile([P, KT1, HW], F8, tag="x", name=f"x_{i}")
                for k in range(KT1):
                    nc.sync.dma_start(x8[:, k, :], x8_d.ap()[i, k])
                    if i == 0:
                        nc.scalar.dma_start(w1t[:, k, :], w1t_d.ap()[k])
                feat = featp.tile([P, KT2, HW], F8, tag="feat")
                with nc.named_scope(f"conv1_{i}"):
                    for nh in range(NH):
                        sl = slice(nh * 512, (nh + 1) * 512)
                        if i == 0 and nh == 1:
                            # deferred consts: loaded while conv1 img0 runs
                            for k in range(KT2):
                                nc.gpsimd.dma_start(qkwt[:, k, :], qkwt_d.ap()[k])
                            for k in range(KT2):
                                nc.gpsimd.dma_start(vwt[:, k, :], vwt_d.ap()[k])
                            for k in range(KT2):
                                nc.gpsimd.dma_start(w3t[:, k, :], w3t_d.ap()[k])
                            nc.gpsimd.dma_start(embt[:], embt_d.ap())
                        for ot in range(OT1):
                            ps = ps_sml.tile([P, 512], F32, tag="mm")
                            for kp in range(0, KT1, 2):
                                nc.tensor.matmul(
                                    ps[:],
                                    w1t[:, kp:kp + 2, ot * P:(ot + 1) * P],
                                    x8[:, kp:kp + 2, sl],
                                    start=(kp == 0), stop=(kp == KT1 - 2),
                                    perf_mode=DR,
                                )
                            # feat = relu(psum/SW + add1/inv1), stored f8
                            nc.scalar.activation(
                                feat[:, ot, sl], ps[:], AF.Relu,
                                scale=1.0 / SW, bias=b1[:, ot:ot + 1])

                # ---- qk (fp8 DR): q=SW*q_true, k=SW*(k_true+emb), f16 ----
                q_sb = qkp.tile([P, HEADS, HW], F16, tag="q")
                k_sb = qkp.tile([P, HEADS, HW], F16, tag="k")
                with nc.named_scope(f"qk_{i}"):
                    for ot in range(2 * HEADS):
                        for nh in range(NH):
                            sl = slice(nh * 512, (nh + 1) * 512)
                            ps = ps_sml.tile([P, 512], F32, tag="mm")
                            for kp in range(0, KT2, 2):
                                nc.tensor.matmul(
                                    ps[:],
                                    qkwt[:, kp:kp + 2, ot * P:(ot + 1) * P],
                                    feat[:, kp:kp + 2, sl],
                                    start=(kp == 0), stop=(kp == KT2 - 2),
                                    perf_mode=DR,
                                )
                            if ot < HEADS:
                                nc.vector.tensor_copy(q_sb[:, ot, sl], ps[:])
                            else:
                                nc.vector.tensor_tensor(
                                    k_sb[:, ot - HEADS, sl], ps[:],
                                    embt[:, sl], ALU.add)

                # ---- attention phase A: logits + exp for ALL heads ----
                # (exp is scalar-bound; keeps lg double-buffered so the PE
                # never waits on an exp read)
                et_tiles = []
                for h in range(HEADS):
                    with nc.named_scope(f"logits_{i}_{h}"):
                        et = etp.tile([P, YT, HW], F8, tag="et",
                                      name=f"et_{i}_{h}")
                        et_tiles.append(et)
                        for yj in range(YT):
                            lg = ps_big.tile([P, 2, 512], F32, tag="lg")
                            for xh in range(NH):
                                nc.tensor.matmul(
                                    lg[:, xh, :],
                                    k_sb[:, h, yj * P:(yj + 1) * P],
                                    q_sb[:, h, xh * 512:(xh + 1) * 512],
                                    start=True, stop=True,
                                )
                            # E = exp(logits) = exp(psum / SW^2), f8
                            nc.scalar.activation(
                                et[:, yj, :].rearrange(
                                    "p (a b) -> p a b", a=2),
                                lg[:, :, :], AF.Exp,
                                scale=1.0 / (SW * SW))

                # ---- v (fp8 DR) -> vaug [y, yt, h, d] f8 = SA*v_true ----
                # (fills the PE while the trailing exps drain)
                vaug = vaugp.tile([P, YT, HEADS, DV], F8, tag="vaug")
                with nc.named_scope(f"v_{i}"):
                    for yt in range(YT):
                        ps = ps_sml.tile([P, 512], F32, tag="mm")
                        for kp in range(0, KT2, 2):
                            nc.tensor.matmul(
                                ps[:],
                                feat[:, kp:kp + 2, yt * P:(yt + 1) * P],
                                vwt[:, kp:kp + 2, :],
                                start=(kp == 0), stop=(kp == KT2 - 2),
                                perf_mode=DR,
                            )
                        nc.vector.tensor_scalar(
                            vaug[:, yt, :, :],
                            ps[:].rearrange("p (h d) -> p h d", d=DV),
                            SA / SW, None, ALU.mult)

                # ---- prefetch residual tiles for conv3 ----
                xres_tiles = []
                for ot in range(OT3):
                    xr_sb = xresp.tile([P, HW], F16, tag="xr",
                                       name=f"xr_{i}_{ot}")
                    nc.scalar.dma_start(xr_sb[:], xr_d.ap()[i, ot])
                    xres_tiles.append(xr_sb)

                # ---- attention phase B: atf[d, x] = SA*softmax(qk)V ----
                atf = atfp.tile([P, KT2, HW], F8, tag="atf")
                for h in range(HEADS):
                    with nc.named_scope(f"attn_{i}_{h}"):
                        et = et_tiles[h]
                        rcb = rcp.tile([P, NH, 512], F32, tag="rcb",
                                       name=f"rcb_{i}_{h}")
                        for xh in range(NH):
                            sl = slice(xh * 512, (xh + 1) * 512)
                            at = ps_sml.tile([P, 512], F32, tag="mm", name="at")
                            sm = ps_sml.tile([P, 512], F32, tag="mm", name="sm")
                            for yp in range(0, YT, 2):
                                nc.tensor.matmul(
                                    at[:],
                                    vaug[:, yp:yp + 2, h, :],
                                    et[:, yp:yp + 2, sl],
                                    start=(yp == 0), stop=(yp == YT - 2),
                                    perf_mode=DR,
                                )
                                # ones stationary is 128 cols wide: the sum
                                # row lands replicated on all partitions for
                                # the same matmul cost (moving-bound)
                                nc.tensor.matmul(
                                    sm[:],
                                    ones2[:, :, :],
                                    et[:, yp:yp + 2, sl],
                                    start=(yp == 0), stop=(yp == YT - 2),
                                    perf_mode=DR,
                                )
                            # rc = 1/S per x; vaug already carries SA
                            nc.vector.reciprocal_approx_fast(rcb[:, xh, :], sm[:])
                            # atf = relu(attnU) * rc  (f8, = SA*attn_true)
                            nc.vector.scalar_tensor_tensor(
                                atf[:, h, sl], at[:], 0.0, rcb[:, xh, :],
                                ALU.max, ALU.mult)

                # ---- conv3 (fp8 DR) + residual(identity mm) + ReLU ----
                with nc.named_scope(f"conv3_{i}"):
                    for ot in range(OT3):
                        o_sb = outp.tile([P, HW], F16, tag="o")
                        for nh in range(NH):
                            sl = slice(nh * 512, (nh + 1) * 512)
                            ps = ps_sml.tile([P, 512], F32, tag="mm")
                            for kp in range(0, KT2, 2):
                                nc.tensor.matmul(
                                    ps[:],
                                    w3t[:, kp:kp + 2, ot * P:(ot + 1) * P],
                                    atf[:, kp:kp + 2, sl],
                                    start=(kp == 0), stop=False,
                                    perf_mode=DR,
                                )
                            # residual: ident*RES @ xres3 accumulates into ps
                            nc.tensor.matmul(
                                ps[:], ident[:], xres_tiles[ot][:, sl],
                                start=False, stop=True)
                            # out = relu(ps / RES), f16
                            if nh == 0:
                                nc.scalar.activation(
                                    o_sb[:, sl], ps[:], AF.Relu,
                                    scale=1.0 / RES)
                            else:
                                nc.vector.tensor_scalar(
                                    o_sb[:, sl], ps[:], 1.0 / RES, 0.0,
                                    ALU.mult, ALU.max)
                        [nc.gpsimd, nc.sync][ot % 2].dma_start(out_d.ap()[i, ot], o_sb[:])

    nc.compile()
    _BUILT["nc"] = nc
    return nc


def _prep_maps(x, conv1_w, gamma1, beta1, mean1, var1, qk_w, v_w, pos_h, pos_w,
               conv3_w, gamma3, beta3, mean3, var3):
    f16 = np.float16
    f32 = np.float32
    inv1 = (gamma1 / np.sqrt(var1 + EPS)).astype(f32)
    add1 = (beta1 - mean1 * inv1).astype(f32)
    inv3 = (gamma3 / np.sqrt(var3 + EPS)).astype(f32)
    add3 = (beta3 - mean3 * inv3).astype(f32)

    def q8(a):
        return np.ascontiguousarray(a).astype(NP_F8)

    w1t = q8((conv1_w.T * SW).reshape(KT1, P, MID))
    # inv1 and the q-scale fold into the qk/v weights; SW for fp8 range
    qk_mod = np.concatenate(
        [qk_w[:HEADS * DQK] * SCALE, qk_w[HEADS * DQK:]], 0) * inv1[None, :]
    qkwt = q8((qk_mod.T * SW).reshape(KT2, P, 2 * MID))
    vwt = q8(((v_w * inv1[None, :]).T * SW).reshape(KT2, P, MID))
    w3t = q8(((conv3_w * inv3[:, None]).T * SW).reshape(KT2, P, CIN))
    embt = np.ascontiguousarray(
        (pos_h[:, None, :] + pos_w[None, :, :]).reshape(HW, DQK).T * SW
    ).astype(f32)
    b1 = np.ascontiguousarray((add1 / inv1).reshape(OT1, P).T).astype(f32)

    xs = x.reshape(N_IMG, KT1, P, HW)
    x8_all = xs.astype(NP_F8)
    xr_all = (xs + add3.reshape(KT1, P)[None, :, :, None]).astype(f16)

    in_maps = []
    for c in range(NCORES):
        sl = slice(c * IMGS_PER_CORE, (c + 1) * IMGS_PER_CORE)
        in_maps.append({
            "x8": np.ascontiguousarray(x8_all[sl]),
            "xr": np.ascontiguousarray(xr_all[sl]),
            "w1t": w1t, "qkwt": qkwt, "vwt": vwt, "w3t": w3t,
            "embt": embt, "b1": b1,
        })
    return in_maps


def kernel(**inputs):
    nc = _build()
    inputs = {k: np.asarray(v) for k, v in inputs.items()}
    in_maps = _prep_maps(**inputs)
    res = run_bass_kernel_spmd(nc, in_maps, core_ids=list(range(NCORES)))
    out = np.concatenate([r["out"] for r in res.results], 0)
    return out.reshape(N_IMG, CIN, H, W).astype(np.float32)
